# revision 1
# baseline (speedup 1.0000x reference)
"""Trainium2 kernel for nn_Decoder (Up-Down captioning decoder).

Strategy (8 NeuronCores, one chip, axon/PJRT):
- One pmap'd XLA program over 8 cores (compiled by neuronx-cc).
- Recurrence is model-parallel: each core owns 128 hidden units (512 gate
  rows) of both LSTMs; full batch (128 examples) on every core; h slices are
  exchanged with jax.lax.all_gather (XLA-native collectives). This keeps the
  per-core recurrent weight slice small enough to stay on-chip.
- Attention is example-parallel: each core handles its 16 examples.
- Phase 1 (feature prep + gate-input preactivations, batched over time) and
  phase 3 (classifier + log_softmax over all steps at once) are
  example/gate-parallel.
- Matmuls run in bf16 with f32 accumulation; LSTM cell math stays f32.
- Host does layout prep only: embedding gather, weight slicing, casts.
"""
import numpy as np
import ml_dtypes
import jax
import jax.numpy as jnp

V = 12000; WE = 1000; FEAT = 2048; FE = 1024; H = 1024; AH = 512
NREG = 36; B = 128; T = 17; NC = 8
BL = B // NC          # examples per core (16)
HL = H // NC          # hidden units per core (128)
GL = 4 * HL           # gate rows per core (512)
TS = T - 1            # time steps (16)

BF = jnp.bfloat16
F32 = jnp.float32


def _mm(a, b):
    return jax.lax.dot(a, b, precision=None, preferred_element_type=F32)


def _forward(args):
    (fc_e_T, attf_T_sh, atte_w, ctx_w, ctx_b, h2att_b,
     wfw_pre_sh, wrec_att_sh, bias_att_sh,
     wrec_lang_sh, bias_lang_sh,
     h2att_w, alpha_w, cls_w, cls_b, w_emb_T) = args

    my = jax.lax.axis_index('i')

    # ---- Phase 1: features (own 16 examples) ----
    # att_e: [BL*NREG, FE] = relu(attf^T @ atte_w)  (atte_b folded on host
    # via ones row in attf_T_sh)
    att_e = jax.nn.relu(_mm(attf_T_sh.T, atte_w))            # [BL*NREG, FE] f32
    p_att = _mm(att_e.astype(BF), ctx_w) + ctx_b             # [BL*NREG, AH] f32

    # pre_att slice for own 512 gates, all (t, b)
    pre_fc = _mm(wfw_pre_sh[:, :FE], fc_e_T)                 # [GL, B]
    pre_w = _mm(wfw_pre_sh[:, FE:], w_emb_T)                 # [GL, TS*B]
    pre_att = (pre_w.reshape(GL, TS, B) + pre_fc[:, None, :]
               + bias_att_sh[:, None, None])                 # [GL, TS, B] f32

    h_att = jnp.zeros((H, B), BF)
    c_att = jnp.zeros((HL, B), F32)
    h_lang = jnp.zeros((H, B), BF)
    c_lang = jnp.zeros((HL, B), F32)

    att_e_b = att_e.reshape(BL, NREG, FE).astype(BF)
    p_att_b = p_att.reshape(BL, NREG, AH)

    def cell(g, c):
        # g: [GL, B] f32 grouped (i, f, gg, o); c: [HL, B] f32
        g = g.reshape(4, HL, B)
        c_new = (jax.nn.sigmoid(g[1]) * c
                 + jax.nn.sigmoid(g[0]) * jnp.tanh(g[2]))
        h_new = jax.nn.sigmoid(g[3]) * jnp.tanh(c_new)
        return h_new, c_new

    hs = []
    for t in range(TS):
        # ---- attention LSTM (gate-sharded, full batch) ----
        x = jnp.concatenate([h_lang, h_att], axis=0)         # [2H, B] bf16
        g = pre_att[:, t, :] + _mm(wrec_att_sh, x)           # [GL, B] f32
        h_att_own, c_att = cell(g, c_att)
        h_att = jax.lax.all_gather(h_att_own.astype(BF), 'i').reshape(H, B)

        # ---- attention (example-sharded: own 16 examples) ----
        h_att_mine = jax.lax.dynamic_slice(h_att, (0, my * BL), (H, BL))
        hq = _mm(h_att_mine.T, h2att_w) + h2att_b            # [BL, AH] f32
        e_in = jnp.tanh(p_att_b + hq[:, None, :])            # [BL, NREG, AH] f32
        e = _mm(e_in.reshape(BL * NREG, AH).astype(BF),
                alpha_w[:, None])[:, 0].reshape(BL, NREG)
        alpha = jax.nn.softmax(e, axis=1)
        # att_res^T own: [FE, BL]
        att_res_own = jnp.einsum('bn,bnd->db', alpha.astype(BF), att_e_b,
                                 preferred_element_type=F32)
        att_res = jax.lax.all_gather(att_res_own.astype(BF), 'i',
                                     axis=1).reshape(FE, B)

        # ---- language LSTM (gate-sharded, full batch) ----
        x = jnp.concatenate([att_res, h_att, h_lang], axis=0)  # [FE+2H, B] bf16
        g = _mm(wrec_lang_sh, x) + bias_lang_sh[:, None]     # [GL, B] f32
        h_lang_own, c_lang = cell(g, c_lang)
        h_lang = jax.lax.all_gather(h_lang_own.astype(BF), 'i').reshape(H, B)

        hs.append(jax.lax.dynamic_slice(h_lang, (0, my * BL), (H, BL)))

    # ---- Phase 3: classifier (own examples, all steps) ----
    hs_flat = jnp.stack(hs, axis=2).reshape(H, BL * TS)      # [H, (b, t)]
    logits = _mm(hs_flat.T, cls_w) + cls_b                   # [(b, t), V] f32
    m = jnp.max(logits, axis=1, keepdims=True)
    lse = m + jnp.log(jnp.sum(jnp.exp(logits - m), axis=1, keepdims=True))
    logp = logits - lse
    return logp.reshape(BL, TS, V)


def prep_args(fc_feats, att_feats, captions, emb_w, fc_w, fc_b, atte_w, atte_b,
              ctx_w, ctx_b, attl_wih, attl_whh, attl_bih, attl_bhh,
              h2att_w, h2att_b, alpha_w, alpha_b,
              langl_wih, langl_whh, langl_bih, langl_bhh, cls_w, cls_b):
    f32 = np.float32
    fc_feats = np.asarray(fc_feats, f32)
    att_feats = np.asarray(att_feats, f32)
    captions = np.asarray(captions)

    # fc_e on host (tiny): [B, FE]
    fc_e = np.maximum(fc_feats @ np.asarray(fc_w, f32) + np.asarray(fc_b, f32), 0.0)
    fc_e_T = np.ascontiguousarray(fc_e.T)                    # [FE, B]

    # embedding gather + relu on host
    its = np.asarray(captions[:, :-1].T)                     # [TS, B]
    w_emb = np.maximum(np.asarray(emb_w, f32)[its], 0.0)     # [TS, B, WE]
    w_emb_T = np.ascontiguousarray(w_emb.reshape(TS * B, WE).T)  # [WE, TS*B]

    # attention-feature embed: fold atte_b via ones-row
    atte_w_aug = np.concatenate([np.asarray(atte_w, f32),
                                 np.asarray(atte_b, f32)[None, :]], axis=0)  # [FEAT+1, FE]

    attl_wih = np.asarray(attl_wih, f32)
    whL_att = attl_wih[:, :H]
    wfw_pre = attl_wih[:, H:]                                # [4H, FE+WE]
    attl_whh = np.asarray(attl_whh, f32)
    bias_att = np.asarray(attl_bih, f32) + np.asarray(attl_bhh, f32)
    # recurrent att weights: x = [h_lang; h_att] -> [4H, 2H]
    wrec_att = np.concatenate([whL_att, attl_whh], axis=1)

    langl_wih = np.asarray(langl_wih, f32)
    langl_whh = np.asarray(langl_whh, f32)
    bias_lang = np.asarray(langl_bih, f32) + np.asarray(langl_bhh, f32)
    # x = [att_res; h_att; h_lang] -> [4H, FE+2H]
    wrec_lang = np.concatenate([langl_wih[:, :FE], langl_wih[:, FE:], langl_whh], axis=1)

    def gate_shard(w):
        w4 = w.reshape(4, H, -1)
        return np.stack([np.ascontiguousarray(
            w4[:, k * HL:(k + 1) * HL].reshape(GL, -1)) for k in range(NC)])

    wrec_att_sh = gate_shard(wrec_att)
    wfw_pre_sh = gate_shard(wfw_pre)
    bias_att_sh = gate_shard(bias_att[:, None])[..., 0]
    wrec_lang_sh = gate_shard(wrec_lang)
    bias_lang_sh = gate_shard(bias_lang[:, None])[..., 0]

    # attention features transposed + ones row, example-sharded
    attf = np.asarray(att_feats, f32).reshape(B * NREG, FEAT)
    attf_aug = np.concatenate([attf, np.ones((B * NREG, 1), f32)], axis=1)
    attf_T = np.ascontiguousarray(attf_aug.T)                # [FEAT+1, B*NREG]
    attf_T_sh = np.stack([np.ascontiguousarray(
        attf_T[:, k * BL * NREG:(k + 1) * BL * NREG]) for k in range(NC)])

    def repl_bf(x):
        a = np.asarray(x, f32).astype(ml_dtypes.bfloat16)
        return np.broadcast_to(a, (NC,) + a.shape)

    def repl_f32(x):
        a = np.asarray(x, f32)
        return np.broadcast_to(a, (NC,) + a.shape)

    def sh_bf(x):
        return np.asarray(x, f32).astype(ml_dtypes.bfloat16)

    args = (
        repl_bf(fc_e_T),
        sh_bf(attf_T_sh),
        repl_bf(atte_w_aug),
        repl_bf(np.asarray(ctx_w, f32)),
        repl_f32(np.asarray(ctx_b, f32)),
        repl_f32(np.asarray(h2att_b, f32)),
        sh_bf(wfw_pre_sh),
        sh_bf(wrec_att_sh),
        bias_att_sh.astype(f32),
        sh_bf(wrec_lang_sh),
        bias_lang_sh.astype(f32),
        repl_bf(h2att_w),
        repl_bf(np.asarray(alpha_w, f32)),
        repl_bf(np.asarray(cls_w, f32)),
        repl_f32(np.asarray(cls_b, f32)),
        repl_bf(w_emb_T),
    )
    return args


def get_fn():
    return jax.pmap(lambda *a: _forward(a), axis_name='i')


def postprocess(out):
    out = np.asarray(out)                           # [NC, BL, TS, V]
    res = np.empty((B, TS, V), np.float32)
    for k in range(NC):
        res[k * BL:(k + 1) * BL] = out[k]
    return res


def kernel(**inputs):
    args = prep_args(**inputs)
    fn = get_fn()
    out = fn(*args)
    return postprocess(out)

